# revision 33
# baseline (speedup 1.0000x reference)
"""MoE gate kernel for TRN2: logits = h @ W.T + bias; softmax; top-2; renorm.

Data-parallel over 8 NeuronCores: B=16384 tokens sharded to 2048/core,
weight (64, 4096) + bias replicated (tiny W split/packed host-side).

Per core (fp16 hi-stream + fp8-DoubleRow lo-stream, d-pair fusion):
  - h loaded naturally [128 tok, 4096 d] (chunks of 256/512 tokens; h
    tiles stream in column-halves, chunk 0 in finer column parts, so
    transposes start as soon as ~128 KB has landed and the DMA ramp
    overlaps compute). Exact fp32 PE-transposes build hT d-PAIR tiles
    [128 d, 2, CHUNK tok] in PSUM (2 banks per tile).
  - Split from PSUM, ONE op per d-pair (the TRN2 read-write-bubble
    errata makes per-op overhead ~172/120 cycles, so fused FD=1024 ops
    beat per-block ones): hhs = fp16(tp * -4096) on ACT (scale=-4096,
    RTN) and hl8 = e4m3(tp*4096 + hhs) on DVE STT (= 4096*(h - hh),
    exact by Sterbenz; e4m3 keeps 4 more bits at 2^-11 scale).
  - S1 (hi): one fp16 matmul per d-block, moving hhs [128, chunk],
    stationary [-Wh16 | -Wl16s] (Wl16s = fp16((W - Wh16)*4096); signs
    cancel hhs' negation). P1[0:64]=4096*hh.Wh, P1[64:128]=2^24*hh.Wl.
  - S2 (lo): one fp8 DoubleRow matmul per d-PAIR (K=256), moving hl8
    [128, 2, chunk], stationary [W8a | W8b] pairs (W8a=e4m3(W),
    W8b=e4m3((W-W8a)*16)); issued before S1 so its larger weight load
    overlaps the transpose train.
  - Combine: lsb = P1lo + (P1hi*2^-12 + 4096*bias) + P2lo + P2hi/16
    (2 ACT + 2 DVE + 1 gpsimd op per chunk; one PSUM input per op).
  - Top-2 per 128-token block (deferred into the next chunk's loop):
    PE back-transpose, DVE max8/idx8, then w1 = sigmoid(-(l2-l1)/4096),
    w2 = 1 - w1 (ACT sigmoid LUT; logits carry a harmless 4096x scale).
  - Outputs accumulate in SBUF; TWO batched DMAs at the end (keeps the
    SP queue free of 32 small descriptor setups).
Max logit err ~3e-5 vs fp64 with 0 top-2 flips on this input (numpy sim;
the old bf16-hi/lo baseline was 2.4e-5 / 0 flips; min top2/3 gap of the
fp32 reference is 2.2e-5). Measured ~170.5 us vs 172.8 us baseline, with
PE-row busy ~121 us (transposes 56 + matmuls 50 + back-transposes), the
ACT/DVE split passes at ~66/78 us (down from 104/93), and HAM staying
warm through all but one 3.4 us window.
"""
import numpy as np
import ml_dtypes
import concourse.bacc as bacc
import concourse.mybir as mybir
from concourse.tile import TileContext
from concourse.bass_utils import run_bass_kernel_spmd
from concourse.masks import make_identity

N_CORES = 8
B = 16384
D = 4096
E = 64
B_SHARD = B // N_CORES      # 2048
DBLK = D // 128              # 32
NPAIR = DBLK // 2            # 16
CHUNKS = [256, 512, 512, 512, 256]
assert sum(CHUNKS) == B_SHARD
NPOST = B_SHARD // 128       # 16
HN_BUFS = 8                  # rolling window of [128, 4096] h tiles

F32 = mybir.dt.float32
F16 = mybir.dt.float16
FP8 = mybir.dt.float8e4
U32 = mybir.dt.uint32
I32 = mybir.dt.int32
AF = mybir.ActivationFunctionType
ALU = mybir.AluOpType
PM = mybir.MatmulPerfMode


def _build():
    nc = bacc.Bacc("TRN2", target_bir_lowering=False, debug=False,
                   num_devices=N_CORES)
    h_d = nc.dram_tensor("h", [B_SHARD, D], F32, kind="ExternalInput")
    w16_d = nc.dram_tensor("w16", [128, DBLK * 128], F16,
                           kind="ExternalInput")
    w8_d = nc.dram_tensor("w8", [128, NPAIR * 2 * 128], FP8,
                          kind="ExternalInput")
    b_d = nc.dram_tensor("bias4096", [E], F32, kind="ExternalInput")
    ow_d = nc.dram_tensor("topk_w", [B_SHARD, 2], F32, kind="ExternalOutput")
    oi_d = nc.dram_tensor("topk_idx", [B_SHARD, 2], I32, kind="ExternalOutput")

    with TileContext(nc) as tc:
        with (
            tc.tile_pool(name="const", bufs=1) as constp,
            tc.tile_pool(name="hnat", bufs=1) as hnatp,
            tc.tile_pool(name="ht", bufs=2) as htp,
            tc.tile_pool(name="small", bufs=2) as smallp,
            tc.tile_pool(name="comb", bufs=1) as combp,
            tc.tile_pool(name="lsbp", bufs=2) as lsbpp,
            tc.tile_pool(name="out", bufs=1) as outp,
            tc.tile_pool(name="tps", bufs=2, space="PSUM") as tpsp,
            tc.tile_pool(name="l1ps", bufs=2, space="PSUM") as l1psp,
            tc.tile_pool(name="l2ps", bufs=1, space="PSUM") as l2psp,
            tc.tile_pool(name="ltps", bufs=1, space="PSUM") as ltpsp,
        ):
            ident = constp.tile([128, 128], F32, name="ident")
            make_identity(nc, ident[:])
            bias_sb = constp.tile([E, 1], F32, name="bias_sb")

            starts = np.cumsum([0] + CHUNKS)
            hn_tiles = {}

            # --- startup: chunk-0 col-part 0, then weights, then the rest
            c0_tiles = CHUNKS[0] // 128
            # per tile: parts covering cols [0:256], [256:1024], then
            # 1024-wide quarters; first part small so transposes start early
            C0_SPANS = [(0, 256), (256, 768), (1024, 1024), (2048, 1024),
                        (3072, 1024)]
            c0_parts = [[None] * len(C0_SPANS) for _ in range(c0_tiles)]

            def load_part(s, q):
                c0_, ln_ = C0_SPANS[q]
                t = hnatp.tile([128, ln_], F32, name=f"hp_{s}_{q}",
                               tag=f"hp_{s}_{q}")
                nc.sync.dma_start(
                    out=t[:], in_=h_d[128 * s:128 * (s + 1), c0_:c0_ + ln_])
                c0_parts[s][q] = t

            for s in range(c0_tiles):
                load_part(s, 0)
            w16 = constp.tile([128, DBLK * 128], F16, name="w16")
            nc.sync.dma_start(out=w16[:], in_=w16_d[:])
            for s in range(c0_tiles):
                load_part(s, 1)
            for s in range(c0_tiles):
                load_part(s, 2)
            w8 = constp.tile([128, NPAIR, 2, 128], FP8, name="w8")
            nc.sync.dma_start(
                out=w8[:], in_=w8_d.ap().rearrange(
                    "p (j k m) -> p j k m", j=NPAIR, k=2))
            nc.sync.dma_start(out=bias_sb[:],
                              in_=b_d.ap().rearrange("(e o) -> e o", o=1))
            for q in range(3, 5):
                for s in range(c0_tiles):
                    load_part(s, q)
            hn_tiles[0] = c0_parts

            def emit_hn_tile(cc, half, s):
                # load column-half `half` of token-tile s of chunk cc;
                # issued staggered so arrivals spread and PE waits stay
                # under the HAM MID window
                base = int(starts[cc])
                if cc not in hn_tiles:
                    hn_tiles[cc] = [[None, None]
                                    for _ in range(CHUNKS[cc] // 128)]
                idx = base // 128 + s
                t = hnatp.tile([128, 2048], F32,
                               name=f"hn_{cc}_{s}_{half}",
                               tag=f"hn_{idx % HN_BUFS}_{half}")
                t0 = base + s * 128
                nc.sync.dma_start(
                    out=t[:],
                    in_=h_d[t0:t0 + 128, 2048 * half:2048 * (half + 1)])
                hn_tiles[cc][s][half] = t

            for s_ in range(CHUNKS[1] // 128):
                emit_hn_tile(1, 0, s_)
            for s_ in range(CHUNKS[1] // 128):
                emit_hn_tile(1, 1, s_)

            ow_all = outp.tile([128, NPOST, 2], F32, name="ow_all")
            oi_all = outp.tile([128, NPOST, 2], I32, name="oi_all")

            pending_posts = []
            pending_combine = []
            for c, chunk in enumerate(CHUNKS):
                tsub = chunk // 128
                tok0 = int(starts[c])
                hn = hn_tiles.pop(c)

                lp1 = l1psp.tile([128, 512], F32, name=f"lp1_{c}", tag="lp1")
                lp2 = l2psp.tile([128, 512], F32, name=f"lp2_{c}", tag="lp2")
                pend = {}
                pend_tp = {}

                def emit_pair(j):
                    # transposes for d-blocks 2j, 2j+1 into one 2-bank tile
                    tp = tpsp.tile([128, 2, 512], F32, name=f"tp_{c}_{j}",
                                   tag="tp")
                    pend_tp[j] = tp
                    hh = htp.tile([128, 2, 512], F16, name=f"hh_{c}_{j}",
                                  tag=f"hh_{j % 3}")
                    hl = htp.tile([128, 2, 512], FP8, name=f"hl_{c}_{j}",
                                  tag=f"hl_{j % 3}")
                    pend[j] = (hh, hl)
                    for k in range(2):
                        d = 2 * j + k
                        for s in range(tsub):
                            if c == 0:
                                col = 128 * d
                                q = next(i for i, (c0_, ln_) in
                                         enumerate([(0, 256), (256, 768),
                                                    (1024, 1024),
                                                    (2048, 1024),
                                                    (3072, 1024)])
                                         if c0_ <= col < c0_ + ln_)
                                c0_ = [0, 256, 1024, 2048, 3072][q]
                                blk = hn[s][q][:, col - c0_:col - c0_ + 128]
                            else:
                                blk = hn[s][d // 16][:, 128 * (d % 16):
                                                     128 * (d % 16 + 1)]
                            nc.tensor.transpose(
                                tp[:, k, 128 * s:128 * (s + 1)], blk,
                                ident[:])
                    emit_split(j)

                def emit_split(j):
                    hh, hl = pend[j]
                    tp = pend_tp[j]
                    with tc.high_priority():
                        nc.scalar.activation(hh[:, :, 0:chunk],
                                             tp[:, :, 0:chunk],
                                             AF.Identity, scale=-4096.0)
                        nc.vector.scalar_tensor_tensor(
                            hl[:, :, 0:chunk], tp[:, :, 0:chunk], 4096.0,
                            hh[:, :, 0:chunk],
                            op0=ALU.mult, op1=ALU.add)

                emit_pair(0)
                emit_pair(1)
                while pending_combine:
                    pending_combine.pop(0)()
                for j in range(NPAIR):
                    if j + 2 < NPAIR:
                        emit_pair(j + 2)
                    if c + 2 < len(CHUNKS):
                        nt = CHUNKS[c + 2] // 128
                        if j == 2:
                            for s_ in range(nt):
                                emit_hn_tile(c + 2, 0, s_)
                        if j == 9:
                            for s_ in range(nt):
                                emit_hn_tile(c + 2, 1, s_)
                    if pending_posts and j >= 3 and (j - 3) % 4 == 0:
                        pending_posts.pop(0)()
                    hh, hl = pend.pop(j)
                    pend_tp.pop(j)
                    nc.tensor.matmul(lp2[:, 0:chunk], w8[:, j],
                                     hl[:, :, 0:chunk],
                                     start=(j == 0), stop=(j == NPAIR - 1),
                                     perf_mode=PM.DoubleRow)
                    for k in range(2):
                        d = 2 * j + k
                        nc.tensor.matmul(lp1[:, 0:chunk],
                                         w16[:, 128 * d:128 * (d + 1)],
                                         hh[:, k, 0:chunk],
                                         start=(d == 0), stop=(d == DBLK - 1))

                # lsb = P1lo + (P1hi*2^-12 + 4096*bias) + P2lo + P2hi/16
                # (emitted deferred, at the start of the NEXT chunk, so the
                # next chunk's casts aren't queued behind these lp-blocked
                # ops in the ACT/DVE FIFOs)
                lsb = lsbpp.tile([E, 512], F32, name=f"lsb_{c}", tag="lsb")

                def make_combine(c, chunk, lp1, lp2, lsb):
                    def combine():
                        aa = combp.tile([E, 512], F32, name=f"aa_{c}",
                                        tag="aa")
                        nc.scalar.activation(aa[:, 0:chunk],
                                             lp2[E:128, 0:chunk],
                                             AF.Identity, scale=0.0625)
                        bb = combp.tile([E, 512], F32, name=f"bb_{c}",
                                        tag="bb")
                        nc.vector.scalar_tensor_tensor(
                            bb[:, 0:chunk], aa[:, 0:chunk], 1.0,
                            lp2[0:E, 0:chunk],
                            op0=ALU.mult, op1=ALU.add)
                        cc_ = combp.tile([E, 512], F32, name=f"cc_{c}",
                                         tag="cc")
                        nc.scalar.activation(cc_[:, 0:chunk],
                                             lp1[E:128, 0:chunk],
                                             AF.Identity, scale=2.0 ** -12,
                                             bias=bias_sb[:])
                        dd_ = combp.tile([E, 512], F32, name=f"dd_{c}",
                                         tag="dd")
                        nc.vector.scalar_tensor_tensor(
                            dd_[:, 0:chunk], cc_[:, 0:chunk], 1.0,
                            lp1[0:E, 0:chunk],
                            op0=ALU.mult, op1=ALU.add)
                        nc.gpsimd.tensor_add(lsb[:, 0:chunk],
                                             bb[:, 0:chunk],
                                             dd_[:, 0:chunk])
                    return combine

                pending_combine.append(make_combine(c, chunk, lp1, lp2, lsb))

                def make_post(c, s, lsb, tokbase):
                    def post():
                        g = tokbase // 128 + s
                        ltp = ltpsp.tile([128, E], F32,
                                         name=f"ltp_{c}_{s}", tag="ltp")
                        nc.tensor.transpose(
                            ltp[:], lsb[:, 128 * s:128 * (s + 1)],
                            ident[0:E, 0:E])
                        lgt = smallp.tile([128, E], F32,
                                          name=f"lgt_{c}_{s}", tag="lgt")
                        nc.scalar.copy(lgt[:], ltp[:])
                        m8 = smallp.tile([128, 8], F32,
                                         name=f"m8_{c}_{s}", tag="m8")
                        i8 = smallp.tile([128, 8], U32,
                                         name=f"i8_{c}_{s}", tag="i8")
                        nc.vector.max_with_indices(m8[:], i8[:], lgt[:])

                        dd = smallp.tile([128, 1], F32,
                                         name=f"dd_{c}_{s}", tag="ddp")
                        nc.gpsimd.tensor_sub(dd[:], m8[:, 1:2], m8[:, 0:1])
                        nc.scalar.activation(ow_all[:, g, 0:1], dd[:],
                                             AF.Sigmoid, scale=-(2.0 ** -12))
                        nc.gpsimd.tensor_scalar(ow_all[:, g, 1:2],
                                                ow_all[:, g, 0:1], -1.0, 1.0,
                                                op0=ALU.mult, op1=ALU.add)
                        nc.gpsimd.tensor_copy(oi_all[:, g, :],
                                              i8[:, 0:2].bitcast(I32))
                    return post

                for s in range(tsub):
                    pending_posts.append(make_post(c, s, lsb, tok0))
            while pending_combine:
                pending_combine.pop(0)()
            for p in pending_posts:
                p()

            nc.sync.dma_start(
                out=ow_d.ap().rearrange("(g p) j -> p g j", p=128),
                in_=ow_all[:])
            nc.sync.dma_start(
                out=oi_d.ap().rearrange("(g p) j -> p g j", p=128),
                in_=oi_all[:])

    nc.compile()
    return nc


_NC = None


def _get_nc():
    global _NC
    if _NC is None:
        _NC = _build()
    return _NC


def _pack_w(weight, bias):
    """Host-side packs (numpy): w16 [128, 32*128] fp16, w8 [128,16*2*128]
    e4m3, bias4096 [64] f32."""
    w64 = weight.astype(np.float64)
    wh16 = weight.astype(np.float16)
    wl16 = ((w64 - wh16.astype(np.float64)) * 4096.0).astype(np.float16)
    w8a = weight.astype(ml_dtypes.float8_e4m3)
    w8b = ((w64 - w8a.astype(np.float64)) * 16.0).astype(np.float32).astype(
        ml_dtypes.float8_e4m3)

    w16 = np.zeros((128, DBLK * 128), dtype=np.float16)
    for d in range(DBLK):
        blk = slice(128 * d, 128 * (d + 1))
        w16[:, 128 * d:128 * d + 64] = -wh16[:, blk].T
        w16[:, 128 * d + 64:128 * (d + 1)] = -wl16[:, blk].T

    w8 = np.zeros((128, NPAIR, 2, 128), dtype=ml_dtypes.float8_e4m3)
    for j in range(NPAIR):
        for k in range(2):
            d = 2 * j + k
            blk = slice(128 * d, 128 * (d + 1))
            w8[:, j, k, 0:64] = w8a[:, blk].T
            w8[:, j, k, 64:128] = w8b[:, blk].T
    b4096 = (bias.astype(np.float64) * 4096.0).astype(np.float32)
    return (np.ascontiguousarray(w16),
            np.ascontiguousarray(w8.reshape(128, NPAIR * 2 * 128)),
            np.ascontiguousarray(b4096))


def run(h, weight, bias, trace=False):
    nc = _get_nc()
    h = np.ascontiguousarray(h, dtype=np.float32)
    weight = np.ascontiguousarray(weight, dtype=np.float32)
    bias = np.ascontiguousarray(bias, dtype=np.float32)
    w16, w8, b4096 = _pack_w(weight, bias)
    in_maps = [{"h": h[i * B_SHARD:(i + 1) * B_SHARD], "w16": w16,
                "w8": w8, "bias4096": b4096} for i in range(N_CORES)]
    res = run_bass_kernel_spmd(nc, in_maps, list(range(N_CORES)), trace=trace)
    tw = np.concatenate([res.results[i]["topk_w"] for i in range(N_CORES)], 0)
    ti = np.concatenate([res.results[i]["topk_idx"] for i in range(N_CORES)], 0)
    return (tw.astype(np.float32), ti.astype(np.int32)), res


def kernel(h, weight, bias):
    (tw, ti), _ = run(h, weight, bias)
    return tw, ti


# revision 34
# speedup vs baseline: 1.0092x; 1.0092x over previous
"""MoE gate kernel for TRN2: logits = h @ W.T + bias; softmax; top-2; renorm.

Data-parallel over 8 NeuronCores: B=16384 tokens sharded to 2048/core,
weight (64, 4096) + bias replicated (tiny W split/packed host-side).

Per core (fp16 hi-stream + fp8-DoubleRow lo-stream, d-pair fusion):
  - h loaded naturally [128 tok, 4096 d] (chunks of 256/512 tokens; h
    tiles stream in column-halves, chunk 0 in finer column parts, so
    transposes start as soon as ~128 KB has landed and the DMA ramp
    overlaps compute). Exact fp32 PE-transposes build hT d-PAIR tiles
    [128 d, 2, CHUNK tok] in PSUM (2 banks per tile).
  - Split from PSUM, ONE op per d-pair (the TRN2 read-write-bubble
    errata makes per-op overhead ~172/120 cycles, so fused FD=1024 ops
    beat per-block ones): hhs = fp16(tp * -4096) on ACT (scale=-4096,
    RTN) and hl8 = e4m3(tp*4096 + hhs) on DVE STT (= 4096*(h - hh),
    exact by Sterbenz; e4m3 keeps 4 more bits at 2^-11 scale).
  - S1 (hi): one fp16 matmul per d-block, moving hhs [128, chunk],
    stationary [-Wh16 | -Wl16s] (Wl16s = fp16((W - Wh16)*4096); signs
    cancel hhs' negation). P1[0:64]=4096*hh.Wh, P1[64:128]=2^24*hh.Wl.
  - S2 (lo): one fp8 DoubleRow matmul per d-PAIR (K=256), moving hl8
    [128, 2, chunk], stationary [W8a | W8b] pairs (W8a=e4m3(W),
    W8b=e4m3((W-W8a)*16)); issued before S1 so its larger weight load
    overlaps the transpose train.
  - Combine: lsb = P1lo + (P1hi*2^-12 + 4096*bias) + P2lo + P2hi/16
    (2 ACT + 2 DVE + 1 gpsimd op per chunk; one PSUM input per op).
  - Top-2 per 128-token block (deferred into the next chunk's loop):
    PE back-transpose, DVE max8/idx8, then w1 = sigmoid(-(l2-l1)/4096),
    w2 = 1 - w1 (ACT sigmoid LUT; logits carry a harmless 4096x scale).
  - Outputs accumulate in SBUF; TWO batched DMAs at the end (keeps the
    SP queue free of 32 small descriptor setups).
Max logit err ~3e-5 vs fp64 with 0 top-2 flips on this input (numpy sim;
the old bf16-hi/lo baseline was 2.4e-5 / 0 flips; min top2/3 gap of the
fp32 reference is 2.2e-5). Measured ~170.5 us vs 172.8 us baseline, with
PE-row busy ~121 us (transposes 56 + matmuls 50 + back-transposes), the
ACT/DVE split passes at ~66/78 us (down from 104/93), and HAM staying
warm through all but one 3.4 us window.
"""
import numpy as np
import ml_dtypes
import concourse.bacc as bacc
import concourse.mybir as mybir
from concourse.tile import TileContext
from concourse.bass_utils import run_bass_kernel_spmd
from concourse.masks import make_identity

N_CORES = 8
B = 16384
D = 4096
E = 64
B_SHARD = B // N_CORES      # 2048
DBLK = D // 128              # 32
NPAIR = DBLK // 2            # 16
CHUNKS = [256, 512, 512, 512, 256]
assert sum(CHUNKS) == B_SHARD
NPOST = B_SHARD // 128       # 16
HN_BUFS = 8                  # rolling window of [128, 4096] h tiles

F32 = mybir.dt.float32
F16 = mybir.dt.float16
FP8 = mybir.dt.float8e4
U32 = mybir.dt.uint32
I32 = mybir.dt.int32
AF = mybir.ActivationFunctionType
ALU = mybir.AluOpType
PM = mybir.MatmulPerfMode


def _build():
    nc = bacc.Bacc("TRN2", target_bir_lowering=False, debug=False,
                   num_devices=N_CORES)
    h_d = nc.dram_tensor("h", [B_SHARD, D], F32, kind="ExternalInput")
    w16_d = nc.dram_tensor("w16", [128, DBLK * 128], F16,
                           kind="ExternalInput")
    w8_d = nc.dram_tensor("w8", [128, NPAIR * 2 * 128], FP8,
                          kind="ExternalInput")
    b_d = nc.dram_tensor("bias4096", [E], F32, kind="ExternalInput")
    ow_d = nc.dram_tensor("topk_w", [B_SHARD, 2], F32, kind="ExternalOutput")
    oi_d = nc.dram_tensor("topk_idx", [B_SHARD, 2], I32, kind="ExternalOutput")

    with TileContext(nc) as tc:
        with (
            tc.tile_pool(name="const", bufs=1) as constp,
            tc.tile_pool(name="hnat", bufs=1) as hnatp,
            tc.tile_pool(name="ht", bufs=2) as htp,
            tc.tile_pool(name="small", bufs=2) as smallp,
            tc.tile_pool(name="comb", bufs=1) as combp,
            tc.tile_pool(name="lsbp", bufs=2) as lsbpp,
            tc.tile_pool(name="out", bufs=1) as outp,
            tc.tile_pool(name="tps", bufs=2, space="PSUM") as tpsp,
            tc.tile_pool(name="l1ps", bufs=2, space="PSUM") as l1psp,
            tc.tile_pool(name="l2ps", bufs=1, space="PSUM") as l2psp,
            tc.tile_pool(name="ltps", bufs=1, space="PSUM") as ltpsp,
        ):
            ident = constp.tile([128, 128], F32, name="ident")
            make_identity(nc, ident[:])
            bias_sb = constp.tile([E, 1], F32, name="bias_sb")

            starts = np.cumsum([0] + CHUNKS)
            hn_tiles = {}

            # --- startup: chunk-0 col-part 0, then weights, then the rest
            c0_tiles = CHUNKS[0] // 128
            # per tile: parts covering cols [0:256], [256:1024], then
            # 1024-wide quarters; first part small so transposes start early
            C0_SPANS = [(0, 256), (256, 768), (1024, 1024), (2048, 1024),
                        (3072, 1024)]
            c0_parts = [[None] * len(C0_SPANS) for _ in range(c0_tiles)]

            def load_part(s, q):
                c0_, ln_ = C0_SPANS[q]
                t = hnatp.tile([128, ln_], F32, name=f"hp_{s}_{q}",
                               tag=f"hp_{s}_{q}")
                nc.sync.dma_start(
                    out=t[:], in_=h_d[128 * s:128 * (s + 1), c0_:c0_ + ln_])
                c0_parts[s][q] = t

            for s in range(c0_tiles):
                load_part(s, 0)
            w16 = constp.tile([128, DBLK * 128], F16, name="w16")
            nc.sync.dma_start(out=w16[:], in_=w16_d[:])
            for s in range(c0_tiles):
                load_part(s, 1)
            for s in range(c0_tiles):
                load_part(s, 2)
            w8 = constp.tile([128, NPAIR, 2, 128], FP8, name="w8")
            nc.sync.dma_start(
                out=w8[:], in_=w8_d.ap().rearrange(
                    "p (j k m) -> p j k m", j=NPAIR, k=2))
            nc.sync.dma_start(out=bias_sb[:],
                              in_=b_d.ap().rearrange("(e o) -> e o", o=1))
            for q in range(3, 5):
                for s in range(c0_tiles):
                    load_part(s, q)
            hn_tiles[0] = c0_parts

            def emit_hn_tile(cc, half, s):
                # load column-half `half` of token-tile s of chunk cc;
                # issued staggered so arrivals spread and PE waits stay
                # under the HAM MID window
                base = int(starts[cc])
                if cc not in hn_tiles:
                    hn_tiles[cc] = [[None, None]
                                    for _ in range(CHUNKS[cc] // 128)]
                idx = base // 128 + s
                t = hnatp.tile([128, 2048], F32,
                               name=f"hn_{cc}_{s}_{half}",
                               tag=f"hn_{idx % HN_BUFS}_{half}")
                t0 = base + s * 128
                nc.sync.dma_start(
                    out=t[:],
                    in_=h_d[t0:t0 + 128, 2048 * half:2048 * (half + 1)])
                hn_tiles[cc][s][half] = t

            for s_ in range(CHUNKS[1] // 128):
                emit_hn_tile(1, 0, s_)
            for s_ in range(CHUNKS[1] // 128):
                emit_hn_tile(1, 1, s_)

            ow_all = outp.tile([128, NPOST, 2], F32, name="ow_all")
            oi_all = outp.tile([128, NPOST, 2], I32, name="oi_all")

            pending_posts = []
            for c, chunk in enumerate(CHUNKS):
                tsub = chunk // 128
                tok0 = int(starts[c])
                hn = hn_tiles.pop(c)

                lp1 = l1psp.tile([128, 512], F32, name=f"lp1_{c}", tag="lp1")
                lp2 = l2psp.tile([128, 512], F32, name=f"lp2_{c}", tag="lp2")
                pend = {}
                pend_tp = {}

                def emit_pair(j):
                    # transposes for d-blocks 2j, 2j+1 into one 2-bank tile
                    tp = tpsp.tile([128, 2, 512], F32, name=f"tp_{c}_{j}",
                                   tag="tp")
                    pend_tp[j] = tp
                    hh = htp.tile([128, 2, 512], F16, name=f"hh_{c}_{j}",
                                  tag=f"hh_{j % 3}")
                    hl = htp.tile([128, 2, 512], FP8, name=f"hl_{c}_{j}",
                                  tag=f"hl_{j % 3}")
                    pend[j] = (hh, hl)
                    for k in range(2):
                        d = 2 * j + k
                        for s in range(tsub):
                            if c == 0:
                                col = 128 * d
                                q = next(i for i, (c0_, ln_) in
                                         enumerate([(0, 256), (256, 768),
                                                    (1024, 1024),
                                                    (2048, 1024),
                                                    (3072, 1024)])
                                         if c0_ <= col < c0_ + ln_)
                                c0_ = [0, 256, 1024, 2048, 3072][q]
                                blk = hn[s][q][:, col - c0_:col - c0_ + 128]
                            else:
                                blk = hn[s][d // 16][:, 128 * (d % 16):
                                                     128 * (d % 16 + 1)]
                            nc.tensor.transpose(
                                tp[:, k, 128 * s:128 * (s + 1)], blk,
                                ident[:])
                    emit_split(j)

                def emit_split(j):
                    hh, hl = pend[j]
                    tp = pend_tp[j]
                    with tc.high_priority():
                        nc.scalar.activation(hh[:, :, 0:chunk],
                                             tp[:, :, 0:chunk],
                                             AF.Identity, scale=-4096.0)
                        nc.vector.scalar_tensor_tensor(
                            hl[:, :, 0:chunk], tp[:, :, 0:chunk], 4096.0,
                            hh[:, :, 0:chunk],
                            op0=ALU.mult, op1=ALU.add)

                emit_pair(0)
                emit_pair(1)
                for j in range(NPAIR):
                    if j + 2 < NPAIR:
                        emit_pair(j + 2)
                    if c + 2 < len(CHUNKS):
                        nt = CHUNKS[c + 2] // 128
                        if j == 2:
                            for s_ in range(nt):
                                emit_hn_tile(c + 2, 0, s_)
                        if j == 9:
                            for s_ in range(nt):
                                emit_hn_tile(c + 2, 1, s_)
                    if pending_posts and j >= 3 and (j - 3) % 4 == 0:
                        pending_posts.pop(0)()
                    hh, hl = pend.pop(j)
                    pend_tp.pop(j)
                    nc.tensor.matmul(lp2[:, 0:chunk], w8[:, j],
                                     hl[:, :, 0:chunk],
                                     start=(j == 0), stop=(j == NPAIR - 1),
                                     perf_mode=PM.DoubleRow)
                    for k in range(2):
                        d = 2 * j + k
                        nc.tensor.matmul(lp1[:, 0:chunk],
                                         w16[:, 128 * d:128 * (d + 1)],
                                         hh[:, k, 0:chunk],
                                         start=(d == 0), stop=(d == DBLK - 1))

                # lsb = P1lo + (P1hi*2^-12 + 4096*bias) + P2lo + P2hi/16
                aa = combp.tile([E, 512], F32, name=f"aa_{c}", tag="aa")
                nc.scalar.activation(aa[:, 0:chunk], lp2[E:128, 0:chunk],
                                     AF.Identity, scale=0.0625)
                bb = combp.tile([E, 512], F32, name=f"bb_{c}", tag="bb")
                nc.vector.scalar_tensor_tensor(
                    bb[:, 0:chunk], aa[:, 0:chunk], 1.0, lp2[0:E, 0:chunk],
                    op0=ALU.mult, op1=ALU.add)
                cc_ = combp.tile([E, 512], F32, name=f"cc_{c}", tag="cc")
                nc.scalar.activation(cc_[:, 0:chunk], lp1[E:128, 0:chunk],
                                     AF.Identity, scale=2.0 ** -12,
                                     bias=bias_sb[:])
                dd_ = combp.tile([E, 512], F32, name=f"dd_{c}", tag="dd")
                nc.vector.scalar_tensor_tensor(
                    dd_[:, 0:chunk], cc_[:, 0:chunk], 1.0, lp1[0:E, 0:chunk],
                    op0=ALU.mult, op1=ALU.add)
                lsb = lsbpp.tile([E, 512], F32, name=f"lsb_{c}", tag="lsb")
                nc.gpsimd.tensor_add(lsb[:, 0:chunk], bb[:, 0:chunk],
                                     dd_[:, 0:chunk])

                def make_post(c, s, lsb, tokbase):
                    def post():
                        g = tokbase // 128 + s
                        ltp = ltpsp.tile([128, E], F32,
                                         name=f"ltp_{c}_{s}", tag="ltp")
                        nc.tensor.transpose(
                            ltp[:], lsb[:, 128 * s:128 * (s + 1)],
                            ident[0:E, 0:E])
                        lgt = smallp.tile([128, E], F32,
                                          name=f"lgt_{c}_{s}", tag="lgt")
                        nc.scalar.copy(lgt[:], ltp[:])
                        m8 = smallp.tile([128, 8], F32,
                                         name=f"m8_{c}_{s}", tag="m8")
                        i8 = smallp.tile([128, 8], U32,
                                         name=f"i8_{c}_{s}", tag="i8")
                        nc.vector.max_with_indices(m8[:], i8[:], lgt[:])

                        dd = smallp.tile([128, 1], F32,
                                         name=f"dd_{c}_{s}", tag="ddp")
                        nc.gpsimd.tensor_sub(dd[:], m8[:, 1:2], m8[:, 0:1])
                        nc.scalar.activation(ow_all[:, g, 0:1], dd[:],
                                             AF.Sigmoid, scale=-(2.0 ** -12))
                        nc.gpsimd.tensor_scalar(ow_all[:, g, 1:2],
                                                ow_all[:, g, 0:1], -1.0, 1.0,
                                                op0=ALU.mult, op1=ALU.add)
                        nc.gpsimd.tensor_copy(oi_all[:, g, :],
                                              i8[:, 0:2].bitcast(I32))
                    return post

                for s in range(tsub):
                    pending_posts.append(make_post(c, s, lsb, tok0))
            for p in pending_posts:
                p()

            nc.sync.dma_start(
                out=ow_d.ap().rearrange("(g p) j -> p g j", p=128),
                in_=ow_all[:])
            nc.sync.dma_start(
                out=oi_d.ap().rearrange("(g p) j -> p g j", p=128),
                in_=oi_all[:])

    nc.compile()
    return nc


_NC = None


def _get_nc():
    global _NC
    if _NC is None:
        _NC = _build()
    return _NC


def _pack_w(weight, bias):
    """Host-side packs (numpy): w16 [128, 32*128] fp16, w8 [128,16*2*128]
    e4m3, bias4096 [64] f32."""
    w64 = weight.astype(np.float64)
    wh16 = weight.astype(np.float16)
    wl16 = ((w64 - wh16.astype(np.float64)) * 4096.0).astype(np.float16)
    w8a = weight.astype(ml_dtypes.float8_e4m3)
    w8b = ((w64 - w8a.astype(np.float64)) * 16.0).astype(np.float32).astype(
        ml_dtypes.float8_e4m3)

    w16 = np.zeros((128, DBLK * 128), dtype=np.float16)
    for d in range(DBLK):
        blk = slice(128 * d, 128 * (d + 1))
        w16[:, 128 * d:128 * d + 64] = -wh16[:, blk].T
        w16[:, 128 * d + 64:128 * (d + 1)] = -wl16[:, blk].T

    w8 = np.zeros((128, NPAIR, 2, 128), dtype=ml_dtypes.float8_e4m3)
    for j in range(NPAIR):
        for k in range(2):
            d = 2 * j + k
            blk = slice(128 * d, 128 * (d + 1))
            w8[:, j, k, 0:64] = w8a[:, blk].T
            w8[:, j, k, 64:128] = w8b[:, blk].T
    b4096 = (bias.astype(np.float64) * 4096.0).astype(np.float32)
    return (np.ascontiguousarray(w16),
            np.ascontiguousarray(w8.reshape(128, NPAIR * 2 * 128)),
            np.ascontiguousarray(b4096))


def run(h, weight, bias, trace=False):
    nc = _get_nc()
    h = np.ascontiguousarray(h, dtype=np.float32)
    weight = np.ascontiguousarray(weight, dtype=np.float32)
    bias = np.ascontiguousarray(bias, dtype=np.float32)
    w16, w8, b4096 = _pack_w(weight, bias)
    in_maps = [{"h": h[i * B_SHARD:(i + 1) * B_SHARD], "w16": w16,
                "w8": w8, "bias4096": b4096} for i in range(N_CORES)]
    res = run_bass_kernel_spmd(nc, in_maps, list(range(N_CORES)), trace=trace)
    tw = np.concatenate([res.results[i]["topk_w"] for i in range(N_CORES)], 0)
    ti = np.concatenate([res.results[i]["topk_idx"] for i in range(N_CORES)], 0)
    return (tw.astype(np.float32), ti.astype(np.int32)), res


def kernel(h, weight, bias):
    (tw, ti), _ = run(h, weight, bias)
    return tw, ti
